# revision 70
# baseline (speedup 1.0000x reference)
"""Trainium2 Bass kernel for nn_GatFeatDecoder (GAT-style decoder).

Reference computation per batch b (B=16, W=64, K=256, E=128, O=64):
    v = x[b].T                               (K, W)
    l = v @ W1.T ; r = v @ W2.T              (K, E) each
    e[i,j]  = sum_e a_e * LeakyReLU(l[i,e] + r[j,e] + lin_b[e]) + bias_kk[i,j]
    attn    = softmax_j(e)
    h       = sigmoid(attn @ v)              (K, W)
    out[b]  = h.T @ fc_w.T + fc_b            (W, O)

Data-parallel: 2 batches per core on 8 cores, no collectives.

Math folding (per-core):
  * z~ = (1-a)|a_e| (l+r+b); sum_e a_e LeakyReLU = sum_e sgn_e relu(z~)
    + alpha' * sum_e sgn_e z~, alpha' = alpha/(1-alpha).  The per-i part
    of the linear term cancels in softmax; the per-j part srb_j =
    sum_w q_w xb[w,j] (q = alpha' * W2b @ sgn, host-folded) is seeded
    into the score PSUM by a rank-1 matmul, so exp() needs no bias.
  * bias_kk^T is accumulated into each score PSUM tile by one extra
    matmul with identity weights; fc_b likewise seeds the fc chain
    (rank-1 fcb x ones), so the post-fc step is a pure copy.
  * softmax without row-max (logits bounded).  attn@v and the
    denominator come from matmuls with rhs = [v | 2.0];
    h = sigmoid(num/den) = 0.5*(tanh(num * (0.5/den)) + 1) via ACT Tanh
    with per-partition scale = reciprocal(2*sum exp); the 0.5/0.5 affine
    is folded into the fc weights/bias on the host.

Score-matmul inversion (the key idea; cost-model timeline 53.1us vs
65.4us for the v1 PE-streaming kernel):
  * v1 streamed each relu tile [E,K] through the PE as the MOVING
    operand of a sign-weighted matmul (256 rows -> ~107ns each, a
    54.6us PE floor for 512 tiles).  Here the relu tile is the
    STATIONARY operand instead: per query-node i the tile
    T^i[e, j] = relu(rtb[e,j] + lt[e,i]) is produced once, and two
    matmuls (one per j-half chain) contract it against a single sgn
    column as the moving operand, writing one PSUM column each.
    Output free size is 1, so each matmul costs ~4ns; the whole score
    reduction is ~2us of PE time and the kernel is bound by relu-tile
    PRODUCTION on DVE/ACT/Pool instead (134/391/451 ns per [128,256]
    tile; DVE runs in 4x mode on bf16).
  * The two j-half score chains are column-INTERLEAVED in one PSUM
    tile (col = 2i+jh) and srb is pre-added by rank-1 matmuls, so one
    bias-free exp() instruction covers both chains per column chunk,
    and chunked exp/attn/tanh overlap the remaining tile production.
  * The PE is nearly idle, so the p-state ramp is irrelevant and v1's
    warm-up matmul prologue is dropped; the input DMA is split into
    dependency-exact chunks and the lt copy is streamed in 32..128
    column pieces so the first tiles start ~1.2us earlier.
  * Production is split DVE/ACT/Pool ~20/6/6 per 32 tiles (inverse to
    the measured per-tile costs), with the fixed per-engine work (exp
    on ACT, copies on DVE + hoisted batch-1 copies on ACT) folded into
    the balance; a few late batch-1 ACT tiles are reassigned to DVE so
    ACT's queue frees for the epilogue tail sooner.
"""

import numpy as np
import ml_dtypes

import concourse.bass as bass
import concourse.bacc as bacc
import concourse.tile as tile
from concourse import mybir
from concourse.bass_utils import run_bass_kernel_spmd

ALPHA = 0.2
B, Wn, K, E, O = 16, 64, 256, 128, 64
N_CORES = 8
BPC = B // N_CORES  # batches per core

FP32 = mybir.dt.float32
BF16 = mybir.dt.bfloat16
AF = mybir.ActivationFunctionType
ALU = mybir.AluOpType

# ---- packed-constant column layout (bf16, 128 partitions) ----
# pack A (early: needed for projections + first relu tiles)
A_W1 = 0                      # w1at   [64,128]  rows 0:64
A_W2 = A_W1 + E               # w2bt   [65,128]  rows 0:65
A_Q = A_W2 + E                # q      [65,1]
A_XB0 = A_Q + 1               # xb b0  [65,256]  rows 0:65 (row 64 = ones)
A_XB1 = A_XB0 + K             # xb b1  [65,256]
A_COLS = A_XB1 + K

# pack B (epilogue constants; lands while batch-0 tiles stream)
B_SGN = 0                     # sgn column [128,1]
B_XTO = B_SGN + 1             # xto2   4 x [128,65]  (b,h) = [v | 2.0]
B_BKT = B_XTO + 4 * (Wn + 1)  # bkkt^T jh-interleaved [128,512] (col 2i+jh)
B_ID = B_BKT + 2 * K          # identity [128,128]
B_FCW = B_ID + E              # fcw2t  2 x [128,64]
B_FCB = B_FCW + 2 * O         # fcb2 row [1,64] (partition 0)
B_ONE = B_FCB + O             # ones row [1,256] (partition 0)
B_COLS = B_ONE + K

# tile-production engine split per 256 i's: DVE 161, ACT 48, Pool 47
# (largest-remainder interleave; inverse to measured per-tile costs
# 134/391/451ns with the fixed per-engine work folded in)
def _make_pattern(n_v=20, n_a=6, n_p=6, span=32):
    quota = {"V": n_v / span, "A": n_a / span, "P": n_p / span}
    acc = {"V": 0.0, "A": 0.0, "P": 0.0}
    pat = []
    for _ in range(span):
        for k in acc:
            acc[k] += quota[k]
        k = max(acc, key=lambda t: acc[t])
        acc[k] -= 1.0
        pat.append(k)
    return pat

PATTERN = _make_pattern()
# surgical engine swaps: lighten ACT at the very end of batch 1 so its
# queue frees for the epilogue tail sooner (counts shift, arrangement kept)
OVERRIDE = {(1, i): "V" for i in range(248, 256) if PATTERN[i % 32] == "A"}


def _build_program():
    nc = bacc.Bacc("TRN2", target_bir_lowering=False, debug=False,
                   num_devices=N_CORES)

    d_packA = nc.dram_tensor("packA", [128, A_COLS], BF16, kind="ExternalInput")
    d_packB = nc.dram_tensor("packB", [128, B_COLS], BF16, kind="ExternalInput")
    d_out = nc.dram_tensor("outp", [O, BPC * Wn], FP32, kind="ExternalOutput")

    with tile.TileContext(nc) as tc:
        with (
            tc.tile_pool(name="consts", bufs=1) as consts,
            tc.tile_pool(name="setup", bufs=2) as setup,
            tc.tile_pool(name="trelu", bufs=96) as trelu,
            tc.tile_pool(name="etiles", bufs=4) as etiles,
            tc.tile_pool(name="small", bufs=8) as small,
            tc.tile_pool(name="psA", bufs=2, space="PSUM") as psA,
            tc.tile_pool(name="psS", bufs=2, space="PSUM") as psS,
            tc.tile_pool(name="psM", bufs=2, space="PSUM") as psM,
            tc.tile_pool(name="psN", bufs=2, space="PSUM") as psN,
        ):
            # separate tiles per DMA chunk so consumers depend only on the
            # chunk they read (whole-tile dep tracking would make the
            # batch-0 projections wait for xb1's DMA too)
            cA1 = consts.tile([128, A_XB1], BF16, tag="cA1")
            nc.sync.dma_start(out=cA1[:], in_=d_packA.ap()[:, 0:A_XB1])
            cA2 = consts.tile([128, K], BF16, tag="cA2")
            nc.sync.dma_start(out=cA2[:], in_=d_packA.ap()[:, A_XB1:A_COLS])
            cB = consts.tile([128, B_COLS], BF16, tag="cB")
            # SP HWDGE (third in queue): keeps the descriptor-gen work off
            # the Pool engine, which is a production bottleneck
            nc.sync.dma_start(out=cB[:], in_=d_packB.ap())

            w1at_v = cA1[0:Wn, A_W1:A_W1 + E]
            w2bt_v = cA1[0:Wn + 1, A_W2:A_W2 + E]
            q_v = cA1[0:Wn + 1, A_Q:A_Q + 1]

            def xb_v(b):
                if b == 0:
                    return cA1[0:Wn + 1, A_XB0:A_XB0 + K]
                return cA2[0:Wn + 1, 0:K]

            sgn_v = cB[:, B_SGN:B_SGN + 1]

            def xto_v(b, h):
                c = B_XTO + (2 * b + h) * (Wn + 1)
                return cB[:, c:c + Wn + 1]

            bkt_v = cB[:, B_BKT:B_BKT + 2 * K]   # jh-interleaved
            ones_v = cB[0:1, B_ONE:B_ONE + K]

            ident_v = cB[:, B_ID:B_ID + E]

            def fcw_v(ih):
                return cB[:, B_FCW + ih * O:B_FCW + (ih + 1) * O]

            fcb_row = cB[0:1, B_FCB:B_FCB + O]
            ot2 = consts.tile([O, BPC * Wn], FP32, tag="ot2")

            # per-batch state
            lt_f = [None] * BPC     # fp32 [E,K]  (scalar/bias source, per-i)
            rtb_b = [None] * BPC    # bf16 [E,K]  (tile in0)
            srb_row = [None] * BPC  # bf16 [1,K]  (srb as a partition-0 row)
            ps_sc = [None] * BPC    # [K//2, 2K] score PSUM, col = 2i+jh
            pT = [None] * BPC       # [K//2, 2K] exp(scores), same layout
            tt = [[None, None] for _ in range(BPC)]
            ps_mix_t = [None] * BPC  # [:,0:65] attn ih0 | [0:64,68:132] fc
            #                          [0:1,132:388] srb row

            def emit_proj(b, chunk_lt=False):
                ps_lr = psA.tile([E, 2 * K], FP32, tag="ps_lr",
                                 name=f"ps_lr{b}")
                # rtb matmul first so its copy (which gates every tile)
                # starts while the lt matmul still runs
                nc.tensor.matmul(ps_lr[:, K:2 * K], w2bt_v, xb_v(b),
                                 start=True, stop=True)
                nc.tensor.matmul(ps_lr[:, 0:K], w1at_v, xb_v(b)[0:Wn, :],
                                 start=True, stop=True)
                lt_f[b] = setup.tile([E, K], FP32, tag="lt_f", name=f"lt_f{b}")
                rtb_b[b] = setup.tile([E, K], BF16, tag="rtb_b",
                                      name=f"rtb_b{b}")
                # batch-0 copies on DVE: rtb first (it gates every tile),
                # then only the first 32 lt columns -- the rest stream in
                # between the first tiles (emit_lt_chunk).  Batch-1 copies
                # go on ACT: the scheduler hoists them into ACT's prologue
                # idle (waiting for the act-table load) instead of delaying
                # DVE's first tile.  (Pool cannot read PSUM at all.)
                if chunk_lt:
                    nc.vector.tensor_copy(rtb_b[b][:], ps_lr[:, K:2 * K])
                    nc.vector.tensor_copy(lt_f[b][:, 0:32], ps_lr[:, 0:32])
                else:
                    nc.scalar.copy(rtb_b[b][:], ps_lr[:, K:2 * K])
                    nc.scalar.copy(lt_f[b][:], ps_lr[:, 0:K])
                # srb row: srb[j] = sum_w q[w] xb[w, j], produced as a
                # [1,256] partition-0 row so it can seed the score PSUM via
                # two rank-1 matmuls (emit_gen_open) -- the exp() then needs
                # no per-partition bias and covers both jh in one instr
                ps_mix_t[b] = psM.tile([128, 390], FP32, tag="ps_mix",
                                       name=f"ps_mix{b}")
                nc.tensor.matmul(ps_mix_t[b][0:1, 132:132 + K], q_v,
                                 xb_v(b), start=True, stop=True)
                srb_row[b] = setup.tile([1, K], BF16, tag="srb_row",
                                        name=f"srb_row{b}")
                nc.scalar.copy(srb_row[b][:], ps_mix_t[b][0:1, 132:132 + K])
                return ps_lr

            def emit_lt_chunk(b, ps_lr, c0, c1):
                nc.vector.tensor_copy(lt_f[b][:, c0:c1], ps_lr[:, c0:c1])

            def emit_gen_open(b):
                ps_sc[b] = psS.tile([K // 2, 2 * K], FP32, tag="ps_sc",
                                    name=f"ps_sc{b}")
                # open both chains at once: bias_kk (jh-interleaved) via
                # identity weights, then srb[j] broadcast along i via one
                # rank-1 matmul per jh (stride-2 output columns)
                nc.tensor.matmul(ps_sc[b][:], ident_v, bkt_v,
                                 start=True, stop=False,
                                 skip_group_check=True)
                for jh in range(2):
                    nc.tensor.matmul(
                        ps_sc[b][:, jh:2 * K:2],
                        srb_row[b][0:1, 128 * jh:128 * jh + 128], ones_v,
                        start=False, stop=False, skip_group_check=True)

            def emit_gen(b, i0, i1):
                for i in range(i0, i1):
                    eng = OVERRIDE.get((b, i)) or PATTERN[i % 32]
                    bias_c = lt_f[b][:, i:i + 1]
                    tj = trelu.tile([E, K], BF16, tag="tj")
                    if eng == "A":
                        nc.scalar.activation(
                            tj[:], rtb_b[b][:], AF.Relu, bias=bias_c,
                            scale=1.0)
                    elif eng == "P":
                        nc.gpsimd.tensor_scalar(
                            out=tj[:], in0=rtb_b[b][:],
                            scalar1=bias_c, scalar2=0.0,
                            op0=ALU.add, op1=ALU.max)
                    else:
                        nc.vector.tensor_scalar(
                            out=tj[:], in0=rtb_b[b][:],
                            scalar1=bias_c, scalar2=0.0,
                            op0=ALU.add, op1=ALU.max)
                    for jh in range(2):
                        nc.tensor.matmul(
                            ps_sc[b][:, 2 * i + jh:2 * i + jh + 1],
                            tj[:, 128 * jh:128 * jh + 128], sgn_v,
                            start=False, stop=(i == K - 1),
                            skip_group_check=True)

            def emit_exp_chunk(b, c0, c1):
                # columns [c0, c1) of both jh chains (interleaved) are
                # final as soon as their col-matmuls have landed; srb is
                # already in PSUM, so one bias-free exp covers both jh
                if pT[b] is None:
                    pT[b] = etiles.tile([K // 2, 2 * K], BF16, tag="pT",
                                        name=f"pT{b}")
                nc.scalar.activation(pT[b][:, 2 * c0:2 * c1],
                                     ps_sc[b][:, 2 * c0:2 * c1], AF.Exp,
                                     scale=1.0)

            attn_num = [[None, None] for _ in range(BPC)]

            def emit_attn_mm(b, ih):
                if ih == 0:
                    num = ps_mix_t[b][:, 0:Wn + 1]
                else:
                    num = psN.tile([K // 2, Wn + 1], FP32, tag="ps_att1",
                                   name=f"ps_att1{b}")
                attn_num[b][ih] = num
                for jh in range(2):
                    nc.tensor.matmul(
                        num,
                        pT[b][:, 256 * ih + jh:256 * ih + jh + 255:2],
                        xto_v(b, jh), start=(jh == 0), stop=(jh == 1))

            def emit_attn_fin(b, ih):
                num = attn_num[b][ih]
                rcol = small.tile([K // 2, 1], FP32, tag=f"rcol{ih}",
                                  name=f"rcol{b}_{ih}")
                nc.vector.reciprocal(rcol[:], num[:, Wn:Wn + 1])
                tt[b][ih] = small.tile([K // 2, Wn], BF16, tag=f"tt{ih}",
                                       name=f"tt{b}_{ih}")
                nc.scalar.activation(tt[b][ih][:], num[:, 0:Wn],
                                     AF.Tanh, scale=rcol[:])

            def emit_fc_bias(b):
                # seed the fc accumulation with fcb (rank-1: fcb ⊗ ones) so
                # no bias-add instruction sits between fc and the out-DMA
                nc.tensor.matmul(ps_mix_t[b][0:O, 68:68 + Wn], fcb_row,
                                 cB[0:1, B_ONE:B_ONE + Wn],
                                 start=True, stop=False,
                                 skip_group_check=True)

            def emit_fc_mm(b):
                ps_o = ps_mix_t[b][0:O, 68:68 + Wn]
                for ih in range(2):
                    nc.tensor.matmul(ps_o, fcw_v(ih), tt[b][ih][:],
                                     start=False, stop=(ih == 1),
                                     skip_group_check=True)

            def emit_fc_out(b, on_act=False):
                # pure PSUM->SBUF copy (fcb already seeded via emit_fc_bias)
                if on_act:
                    nc.scalar.copy(ot2[:, Wn * b:Wn * (b + 1)],
                                   ps_mix_t[b][0:O, 68:68 + Wn])
                else:
                    nc.vector.tensor_copy(ot2[:, Wn * b:Wn * (b + 1)],
                                          ps_mix_t[b][0:O, 68:68 + Wn])

            # software-pipelined emission: epilogue pieces are interleaved
            # into the tile stream so in-order engine queues never block on
            # a not-yet-ready epilogue instruction (emission position is
            # queue position), and only a short chain trails the last tile
            ps_lr0 = emit_proj(0, chunk_lt=True)
            emit_gen_open(0)
            emit_gen(0, 0, 8)
            emit_lt_chunk(0, ps_lr0, 32, 64)
            emit_gen(0, 8, 20)
            emit_lt_chunk(0, ps_lr0, 64, 128)
            emit_gen(0, 20, 40)
            emit_lt_chunk(0, ps_lr0, 128, K)
            emit_gen(0, 40, 96)
            emit_proj(1)
            emit_gen(0, 96, 132)
            emit_exp_chunk(0, 0, 128)
            emit_gen(0, 132, 140)
            emit_attn_mm(0, 0)
            emit_fc_bias(0)
            emit_gen(0, 140, 164)
            emit_attn_fin(0, 0)
            emit_gen(0, 164, K)
            emit_gen_open(1)
            emit_gen(1, 0, 4)
            emit_exp_chunk(0, 128, K)   # b0 cols [128:256)
            emit_gen(1, 4, 12)
            emit_attn_mm(0, 1)
            emit_gen(1, 12, 36)
            emit_attn_fin(0, 1)
            emit_gen(1, 36, 44)
            emit_fc_mm(0)
            emit_fc_out(0)
            nc.sync.dma_start(out=d_out.ap()[:, 0:Wn], in_=ot2[:, 0:Wn])
            emit_gen(1, 44, 132)
            emit_exp_chunk(1, 0, 128)
            emit_gen(1, 132, 140)
            emit_attn_mm(1, 0)
            emit_fc_bias(1)
            emit_gen(1, 140, 164)
            emit_attn_fin(1, 0)
            emit_gen(1, 164, 196)
            emit_exp_chunk(1, 128, 192)
            emit_gen(1, 196, K)
            emit_exp_chunk(1, 192, K)   # b1 tail chunk (small)
            emit_attn_mm(1, 1)
            emit_attn_fin(1, 1)
            emit_fc_mm(1)
            emit_fc_out(1, on_act=True)
            nc.sync.dma_start(out=d_out.ap()[:, Wn:2 * Wn],
                              in_=ot2[:, Wn:2 * Wn])

    nc.compile()
    return nc


_NC_CACHE = {}


def _get_program():
    if "nc" not in _NC_CACHE:
        _NC_CACHE["nc"] = _build_program()
    return _NC_CACHE["nc"]


def _host_prep(x, lin_w, lin_b, a, bias_kk, fc_w, fc_b):
    f32 = np.float32
    bf16 = ml_dtypes.bfloat16
    x = np.ascontiguousarray(x, f32)
    aa = (np.abs(a) * (1.0 - ALPHA)).astype(f32)
    sgn = np.sign(a).astype(f32)
    alpha_p = ALPHA / (1.0 - ALPHA)

    w1at = (lin_w[:, :Wn] * aa[:, None]).T.astype(f32)          # [64,128]
    w2t = (lin_w[:, Wn:] * aa[:, None]).T                        # [64,128]
    bt = (lin_b * aa)[None, :]
    w2bt = np.concatenate([w2t, bt], 0).astype(f32)              # [65,128]
    q = (alpha_p * (w2bt @ sgn)).astype(f32)                     # [65]
    bkkt = bias_kk.T.astype(f32)                                 # [256,256]
    fcw2t = (0.5 * fc_w).T.astype(f32)                           # [256,64]
    fcb2 = (fc_b + 0.5 * fc_w.sum(1)).astype(f32)                # [64]

    packA = np.zeros((128, A_COLS), f32)
    packA[0:Wn, A_W1:A_W1 + E] = w1at
    packA[0:Wn + 1, A_W2:A_W2 + E] = w2bt
    packA[0:Wn + 1, A_Q] = q

    packB_shared = np.zeros((128, B_COLS), f32)
    packB_shared[:, B_SGN] = sgn
    # jh-interleaved: col 2i+jh holds bkkt[jh*128:(jh+1)*128, i]
    packB_shared[:, B_BKT:B_BKT + 2 * K] = np.transpose(
        bkkt.reshape(2, 128, K), (1, 2, 0)).reshape(128, 2 * K)
    packB_shared[0:1, B_ONE:B_ONE + K] = 1.0
    packB_shared[:, B_ID:B_ID + E] = np.eye(128, dtype=f32)
    packB_shared[:, B_FCW:B_FCW + O] = fcw2t[0:128, :]
    packB_shared[:, B_FCW + O:B_FCW + 2 * O] = fcw2t[128:256, :]
    packB_shared[0, B_FCB:B_FCB + O] = fcb2

    in_maps = []
    for c in range(N_CORES):
        pa = packA.copy()
        pb = packB_shared.copy()
        for i in range(BPC):
            xb = x[BPC * c + i]                                  # [64,256]
            xb1 = np.concatenate([xb, np.ones((1, K), f32)], 0)  # [65,256]
            vt = xb.T                                            # [256,64]
            xto2 = np.concatenate([vt, np.full((K, 1), 2.0, f32)], 1)
            col = A_XB0 if i == 0 else A_XB1
            pa[0:Wn + 1, col:col + K] = xb1
            for h in range(2):
                c0 = B_XTO + (2 * i + h) * (Wn + 1)
                pb[:, c0:c0 + Wn + 1] = xto2[128 * h:128 * h + 128, :]
        in_maps.append({
            "packA": np.ascontiguousarray(pa.astype(bf16)),
            "packB": np.ascontiguousarray(pb.astype(bf16)),
        })
    return in_maps


def kernel(x, lin_w, lin_b, a, bias_kk, fc_w, fc_b, _trace=False):
    nc = _get_program()
    in_maps = _host_prep(np.asarray(x), np.asarray(lin_w), np.asarray(lin_b),
                         np.asarray(a), np.asarray(bias_kk),
                         np.asarray(fc_w), np.asarray(fc_b))
    res = run_bass_kernel_spmd(nc, in_maps, list(range(N_CORES)),
                               trace=_trace)
    out = np.empty((B, Wn, O), np.float32)
    for c in range(N_CORES):
        o = res.results[c]["outp"]          # (O, BPC*Wn)
        for i in range(BPC):
            out[BPC * c + i] = o[:, Wn * i:Wn * (i + 1)].T
    if _trace:
        return out, res
    return out


# revision 76
# speedup vs baseline: 1.0014x; 1.0014x over previous
"""Trainium2 Bass kernel for nn_GatFeatDecoder (GAT-style decoder).

Reference computation per batch b (B=16, W=64, K=256, E=128, O=64):
    v = x[b].T                               (K, W)
    l = v @ W1.T ; r = v @ W2.T              (K, E) each
    e[i,j]  = sum_e a_e * LeakyReLU(l[i,e] + r[j,e] + lin_b[e]) + bias_kk[i,j]
    attn    = softmax_j(e)
    h       = sigmoid(attn @ v)              (K, W)
    out[b]  = h.T @ fc_w.T + fc_b            (W, O)

Data-parallel: 2 batches per core on 8 cores, no collectives.

Math folding (per-core), same as the v1 kernel:
  * z~ = (1-a)|a_e| (l+r+b); sum_e a_e LeakyReLU = sum_e sgn_e relu(z~)
    + alpha' * sum_e sgn_e z~, alpha' = alpha/(1-alpha).  The per-i part
    of the linear term cancels in softmax; the per-j part srb_j =
    sum_w q_w xb[w,j] with q = alpha' * W2b @ sgn precomputed on device,
    and enters as the per-partition bias of the exp() activation.
  * bias_kk^T is accumulated into each score PSUM tile by one extra
    matmul with identity weights, so exp() reads PSUM directly.
  * softmax without row-max (logits bounded): P^T = exp(S^T + srb_j).
    attn@v and the denominator come from matmuls with rhs = [v | 2.0];
    h = sigmoid(num/den) = 0.5*(tanh(num * (0.5/den)) + 1) via ACT Tanh
    with per-partition scale = reciprocal(2*sum exp); the 0.5/0.5 affine
    is folded into the fc weights/bias on the host.

v2 score-matmul inversion (the big change vs v1):
  * v1 streamed each relu tile [E,K] through the PE as the MOVING
    operand of a sign-weighted matmul (256 rows -> ~107ns each, a
    54.6us PE floor for 512 tiles).  v2 makes the relu tile the
    STATIONARY operand instead: per query-node i the tile
    T^i[e, j] = relu(rtb[e,j] + lt[e,i]) is produced once, and two
    matmuls (one per j-half chain) contract it against a single sgn
    column as the moving operand, writing one PSUM column
    S^T[jh][:, i].  Output free size is 1, so each matmul costs ~4ns;
    the whole score reduction is ~2us of PE time and the kernel is
    bound by relu-tile PRODUCTION on DVE/ACT/Pool instead
    (134/391/429 ns per [128,256] tile; DVE runs in 4x mode on bf16).
  * Tiles are indexed by i (bias = lt column) rather than j (bias =
    rtb column) so S^T lands in the same [j-half, i] layout v1 used;
    the exp/attn/fc epilogue is unchanged.
  * The PE is now nearly idle, so the p-state ramp is irrelevant and
    v1's warm-up matmul prologue is dropped entirely.
  * Production is split DVE/ACT/Pool ~20/6/6 per 32 tiles (inverse to
    the measured per-tile costs), with the fixed per-engine work (exp
    on ACT, copies on DVE/Pool) folded into the balance.
"""

import numpy as np
import ml_dtypes

import concourse.bass as bass
import concourse.bacc as bacc
import concourse.tile as tile
from concourse import mybir
from concourse.bass_utils import run_bass_kernel_spmd

ALPHA = 0.2
B, Wn, K, E, O = 16, 64, 256, 128, 64
N_CORES = 8
BPC = B // N_CORES  # batches per core

FP32 = mybir.dt.float32
BF16 = mybir.dt.bfloat16
AF = mybir.ActivationFunctionType
ALU = mybir.AluOpType

# ---- packed-constant column layout (bf16, 128 partitions) ----
# pack A (early: needed for projections + first relu tiles)
A_W1 = 0                      # w1at   [64,128]  rows 0:64
A_W2 = A_W1 + E               # w2bt   [65,128]  rows 0:65
A_Q = A_W2 + E                # q      [65,1]
A_XB0 = A_Q + 1               # xb b0  [65,256]  rows 0:65 (row 64 = ones)
A_XB1 = A_XB0 + K             # xb b1  [65,256]
A_COLS = A_XB1 + K

# pack B (epilogue constants; lands while batch-0 tiles stream)
B_SGN = 0                     # sgn column [128,1]
B_XTO = B_SGN + 1             # xto2   4 x [128,65]  (b,h) = [v | 2.0]
B_BKT = B_XTO + 4 * (Wn + 1)  # bkkt^T jh-interleaved [128,512] (col 2i+jh)
B_ID = B_BKT + 2 * K          # identity [128,128]
B_FCW = B_ID + E              # fcw2t  2 x [128,64]
B_FCB = B_FCW + 2 * O         # fcb2 row [1,64] (partition 0)
B_ONE = B_FCB + O             # ones row [1,256] (partition 0)
B_COLS = B_ONE + K

# tile-production engine split per 256 i's: DVE 161, ACT 48, Pool 47
# (largest-remainder interleave; inverse to measured per-tile costs
# 134/391/451ns with the fixed per-engine work folded in)
def _make_pattern(n_v=20, n_a=6, n_p=6, span=32):
    quota = {"V": n_v / span, "A": n_a / span, "P": n_p / span}
    acc = {"V": 0.0, "A": 0.0, "P": 0.0}
    pat = []
    for _ in range(span):
        for k in acc:
            acc[k] += quota[k]
        k = max(acc, key=lambda t: acc[t])
        acc[k] -= 1.0
        pat.append(k)
    return pat

PATTERN = _make_pattern()
# surgical engine swaps: lighten ACT at the very end of batch 1 so its
# queue frees for the epilogue tail sooner (counts shift, arrangement kept)
OVERRIDE = {(1, i): "V" for i in range(248, 256) if PATTERN[i % 32] == "A"}


def _build_program():
    nc = bacc.Bacc("TRN2", target_bir_lowering=False, debug=False,
                   num_devices=N_CORES)

    d_packA = nc.dram_tensor("packA", [128, A_COLS], BF16, kind="ExternalInput")
    d_packB = nc.dram_tensor("packB", [128, B_COLS], BF16, kind="ExternalInput")
    d_out = nc.dram_tensor("outp", [O, BPC * Wn], FP32, kind="ExternalOutput")

    with tile.TileContext(nc) as tc:
        with (
            tc.tile_pool(name="consts", bufs=1) as consts,
            tc.tile_pool(name="setup", bufs=2) as setup,
            tc.tile_pool(name="trelu", bufs=96) as trelu,
            tc.tile_pool(name="etiles", bufs=4) as etiles,
            tc.tile_pool(name="small", bufs=8) as small,
            tc.tile_pool(name="psA", bufs=2, space="PSUM") as psA,
            tc.tile_pool(name="psS", bufs=2, space="PSUM") as psS,
            tc.tile_pool(name="psM", bufs=2, space="PSUM") as psM,
            tc.tile_pool(name="psN", bufs=2, space="PSUM") as psN,
        ):
            # separate tiles per DMA chunk so consumers depend only on the
            # chunk they read (whole-tile dep tracking would make the
            # batch-0 projections wait for xb1's DMA too)
            cA1 = consts.tile([128, A_XB1], BF16, tag="cA1")
            nc.sync.dma_start(out=cA1[:], in_=d_packA.ap()[:, 0:A_XB1])
            cA2 = consts.tile([128, K], BF16, tag="cA2")
            nc.sync.dma_start(out=cA2[:], in_=d_packA.ap()[:, A_XB1:A_COLS])
            cB = consts.tile([128, B_COLS], BF16, tag="cB")
            # SP HWDGE (third in queue): keeps the descriptor-gen work off
            # the Pool engine, which is a production bottleneck
            nc.sync.dma_start(out=cB[:], in_=d_packB.ap())

            w1at_v = cA1[0:Wn, A_W1:A_W1 + E]
            w2bt_v = cA1[0:Wn + 1, A_W2:A_W2 + E]
            q_v = cA1[0:Wn + 1, A_Q:A_Q + 1]

            def xb_v(b):
                if b == 0:
                    return cA1[0:Wn + 1, A_XB0:A_XB0 + K]
                return cA2[0:Wn + 1, 0:K]

            sgn_v = cB[:, B_SGN:B_SGN + 1]

            def xto_v(b, h):
                c = B_XTO + (2 * b + h) * (Wn + 1)
                return cB[:, c:c + Wn + 1]

            bkt_v = cB[:, B_BKT:B_BKT + 2 * K]   # jh-interleaved
            ones_v = cB[0:1, B_ONE:B_ONE + K]

            ident_v = cB[:, B_ID:B_ID + E]

            def fcw_v(ih):
                return cB[:, B_FCW + ih * O:B_FCW + (ih + 1) * O]

            fcb_row = cB[0:1, B_FCB:B_FCB + O]
            ot2 = consts.tile([O, BPC * Wn], FP32, tag="ot2")

            # per-batch state
            lt_f = [None] * BPC     # fp32 [E,K]  (scalar/bias source, per-i)
            rtb_b = [None] * BPC    # bf16 [E,K]  (tile in0)
            srb_row = [None] * BPC  # bf16 [1,K]  (srb as a partition-0 row)
            ps_sc = [None] * BPC    # [K//2, 2K] score PSUM, col = 2i+jh
            pT = [None] * BPC       # [K//2, 2K] exp(scores), same layout
            tt = [[None, None] for _ in range(BPC)]
            ps_mix_t = [None] * BPC  # [:,0:65] attn ih0 | [0:64,68:132] fc
            #                          [0:1,132:388] srb row

            def emit_proj(b, chunk_lt=False):
                ps_lr = psA.tile([E, 2 * K], FP32, tag="ps_lr",
                                 name=f"ps_lr{b}")
                # lt matmul first: the 32-column lt chunk plus rtb are
                # what the first tiles need; this order lets the lt chunk
                # copy overlap the rtb matmul
                nc.tensor.matmul(ps_lr[:, 0:K], w1at_v, xb_v(b)[0:Wn, :],
                                 start=True, stop=True)
                nc.tensor.matmul(ps_lr[:, K:2 * K], w2bt_v, xb_v(b),
                                 start=True, stop=True)
                lt_f[b] = setup.tile([E, K], FP32, tag="lt_f", name=f"lt_f{b}")
                rtb_b[b] = setup.tile([E, K], BF16, tag="rtb_b",
                                      name=f"rtb_b{b}")
                # batch-0 copies on DVE: rtb first (it gates every tile),
                # then only the first 32 lt columns -- the rest stream in
                # between the first tiles (emit_lt_chunk).  Batch-1 copies
                # go on ACT: the scheduler hoists them into ACT's prologue
                # idle (waiting for the act-table load) instead of delaying
                # DVE's first tile.  (Pool cannot read PSUM at all.)
                if chunk_lt:
                    nc.vector.tensor_copy(lt_f[b][:, 0:32], ps_lr[:, 0:32])
                    nc.vector.tensor_copy(rtb_b[b][:], ps_lr[:, K:2 * K])
                else:
                    nc.scalar.copy(rtb_b[b][:], ps_lr[:, K:2 * K])
                    nc.scalar.copy(lt_f[b][:], ps_lr[:, 0:K])
                # srb row: srb[j] = sum_w q[w] xb[w, j], produced as a
                # [1,256] partition-0 row so it can seed the score PSUM via
                # two rank-1 matmuls (emit_gen_open) -- the exp() then needs
                # no per-partition bias and covers both jh in one instr
                ps_mix_t[b] = psM.tile([128, 390], FP32, tag="ps_mix",
                                       name=f"ps_mix{b}")
                nc.tensor.matmul(ps_mix_t[b][0:1, 132:132 + K], q_v,
                                 xb_v(b), start=True, stop=True)
                srb_row[b] = setup.tile([1, K], BF16, tag="srb_row",
                                        name=f"srb_row{b}")
                nc.scalar.copy(srb_row[b][:], ps_mix_t[b][0:1, 132:132 + K])
                return ps_lr

            def emit_lt_chunk(b, ps_lr, c0, c1):
                nc.vector.tensor_copy(lt_f[b][:, c0:c1], ps_lr[:, c0:c1])

            def emit_gen_open(b):
                ps_sc[b] = psS.tile([K // 2, 2 * K], FP32, tag="ps_sc",
                                    name=f"ps_sc{b}")
                # open both chains at once: bias_kk (jh-interleaved) via
                # identity weights, then srb[j] broadcast along i via one
                # rank-1 matmul per jh (stride-2 output columns)
                nc.tensor.matmul(ps_sc[b][:], ident_v, bkt_v,
                                 start=True, stop=False,
                                 skip_group_check=True)
                for jh in range(2):
                    nc.tensor.matmul(
                        ps_sc[b][:, jh:2 * K:2],
                        srb_row[b][0:1, 128 * jh:128 * jh + 128], ones_v,
                        start=False, stop=False, skip_group_check=True)

            def emit_gen(b, i0, i1):
                for i in range(i0, i1):
                    eng = OVERRIDE.get((b, i)) or PATTERN[i % 32]
                    bias_c = lt_f[b][:, i:i + 1]
                    tj = trelu.tile([E, K], BF16, tag="tj")
                    if eng == "A":
                        nc.scalar.activation(
                            tj[:], rtb_b[b][:], AF.Relu, bias=bias_c,
                            scale=1.0)
                    elif eng == "P":
                        nc.gpsimd.tensor_scalar(
                            out=tj[:], in0=rtb_b[b][:],
                            scalar1=bias_c, scalar2=0.0,
                            op0=ALU.add, op1=ALU.max)
                    else:
                        nc.vector.tensor_scalar(
                            out=tj[:], in0=rtb_b[b][:],
                            scalar1=bias_c, scalar2=0.0,
                            op0=ALU.add, op1=ALU.max)
                    for jh in range(2):
                        nc.tensor.matmul(
                            ps_sc[b][:, 2 * i + jh:2 * i + jh + 1],
                            tj[:, 128 * jh:128 * jh + 128], sgn_v,
                            start=False, stop=(i == K - 1),
                            skip_group_check=True)

            def emit_exp_chunk(b, c0, c1):
                # columns [c0, c1) of both jh chains (interleaved) are
                # final as soon as their col-matmuls have landed; srb is
                # already in PSUM, so one bias-free exp covers both jh
                if pT[b] is None:
                    pT[b] = etiles.tile([K // 2, 2 * K], BF16, tag="pT",
                                        name=f"pT{b}")
                nc.scalar.activation(pT[b][:, 2 * c0:2 * c1],
                                     ps_sc[b][:, 2 * c0:2 * c1], AF.Exp,
                                     scale=1.0)

            attn_num = [[None, None] for _ in range(BPC)]

            def emit_attn_mm(b, ih):
                if ih == 0:
                    num = ps_mix_t[b][:, 0:Wn + 1]
                else:
                    num = psN.tile([K // 2, Wn + 1], FP32, tag="ps_att1",
                                   name=f"ps_att1{b}")
                attn_num[b][ih] = num
                for jh in range(2):
                    nc.tensor.matmul(
                        num,
                        pT[b][:, 256 * ih + jh:256 * ih + jh + 255:2],
                        xto_v(b, jh), start=(jh == 0), stop=(jh == 1))

            def emit_attn_fin(b, ih):
                num = attn_num[b][ih]
                rcol = small.tile([K // 2, 1], FP32, tag=f"rcol{ih}",
                                  name=f"rcol{b}_{ih}")
                nc.vector.reciprocal(rcol[:], num[:, Wn:Wn + 1])
                tt[b][ih] = small.tile([K // 2, Wn], BF16, tag=f"tt{ih}",
                                       name=f"tt{b}_{ih}")
                nc.scalar.activation(tt[b][ih][:], num[:, 0:Wn],
                                     AF.Tanh, scale=rcol[:])

            def emit_fc_bias(b):
                # seed the fc accumulation with fcb (rank-1: fcb ⊗ ones) so
                # no bias-add instruction sits between fc and the out-DMA
                nc.tensor.matmul(ps_mix_t[b][0:O, 68:68 + Wn], fcb_row,
                                 cB[0:1, B_ONE:B_ONE + Wn],
                                 start=True, stop=False,
                                 skip_group_check=True)

            def emit_fc_mm(b):
                ps_o = ps_mix_t[b][0:O, 68:68 + Wn]
                for ih in range(2):
                    nc.tensor.matmul(ps_o, fcw_v(ih), tt[b][ih][:],
                                     start=False, stop=(ih == 1),
                                     skip_group_check=True)

            def emit_fc_out(b, on_act=False):
                # pure PSUM->SBUF copy (fcb already seeded via emit_fc_bias)
                if on_act:
                    nc.scalar.copy(ot2[:, Wn * b:Wn * (b + 1)],
                                   ps_mix_t[b][0:O, 68:68 + Wn])
                else:
                    nc.vector.tensor_copy(ot2[:, Wn * b:Wn * (b + 1)],
                                          ps_mix_t[b][0:O, 68:68 + Wn])

            # software-pipelined emission: epilogue pieces are interleaved
            # into the tile stream so in-order engine queues never block on
            # a not-yet-ready epilogue instruction (emission position is
            # queue position), and only a short chain trails the last tile
            ps_lr0 = emit_proj(0, chunk_lt=True)
            emit_gen_open(0)
            emit_gen(0, 0, 8)
            emit_lt_chunk(0, ps_lr0, 32, 64)
            emit_gen(0, 8, 20)
            emit_lt_chunk(0, ps_lr0, 64, 128)
            emit_gen(0, 20, 40)
            emit_lt_chunk(0, ps_lr0, 128, K)
            emit_gen(0, 40, 96)
            emit_proj(1)
            emit_gen(0, 96, 132)
            emit_exp_chunk(0, 0, 128)
            emit_gen(0, 132, 140)
            emit_attn_mm(0, 0)
            emit_fc_bias(0)
            emit_gen(0, 140, 164)
            emit_attn_fin(0, 0)
            emit_gen(0, 164, K)
            emit_gen_open(1)
            emit_gen(1, 0, 4)
            emit_exp_chunk(0, 128, K)   # b0 cols [128:256)
            emit_gen(1, 4, 12)
            emit_attn_mm(0, 1)
            emit_gen(1, 12, 36)
            emit_attn_fin(0, 1)
            emit_gen(1, 36, 44)
            emit_fc_mm(0)
            emit_fc_out(0)
            nc.sync.dma_start(out=d_out.ap()[:, 0:Wn], in_=ot2[:, 0:Wn])
            emit_gen(1, 44, 132)
            emit_exp_chunk(1, 0, 128)
            emit_gen(1, 132, 140)
            emit_attn_mm(1, 0)
            emit_fc_bias(1)
            emit_gen(1, 140, 164)
            emit_attn_fin(1, 0)
            emit_gen(1, 164, 196)
            emit_exp_chunk(1, 128, 192)
            emit_gen(1, 196, 244)
            emit_exp_chunk(1, 192, 240)
            emit_gen(1, 244, K)
            emit_exp_chunk(1, 240, K)   # b1 tail chunk (tiny)
            emit_attn_mm(1, 1)
            emit_attn_fin(1, 1)
            emit_fc_mm(1)
            emit_fc_out(1, on_act=True)
            nc.sync.dma_start(out=d_out.ap()[:, Wn:2 * Wn],
                              in_=ot2[:, Wn:2 * Wn])

    nc.compile()
    return nc


_NC_CACHE = {}


def _get_program():
    if "nc" not in _NC_CACHE:
        _NC_CACHE["nc"] = _build_program()
    return _NC_CACHE["nc"]


def _host_prep(x, lin_w, lin_b, a, bias_kk, fc_w, fc_b):
    f32 = np.float32
    bf16 = ml_dtypes.bfloat16
    x = np.ascontiguousarray(x, f32)
    aa = (np.abs(a) * (1.0 - ALPHA)).astype(f32)
    sgn = np.sign(a).astype(f32)
    alpha_p = ALPHA / (1.0 - ALPHA)

    w1at = (lin_w[:, :Wn] * aa[:, None]).T.astype(f32)          # [64,128]
    w2t = (lin_w[:, Wn:] * aa[:, None]).T                        # [64,128]
    bt = (lin_b * aa)[None, :]
    w2bt = np.concatenate([w2t, bt], 0).astype(f32)              # [65,128]
    q = (alpha_p * (w2bt @ sgn)).astype(f32)                     # [65]
    bkkt = bias_kk.T.astype(f32)                                 # [256,256]
    fcw2t = (0.5 * fc_w).T.astype(f32)                           # [256,64]
    fcb2 = (fc_b + 0.5 * fc_w.sum(1)).astype(f32)                # [64]

    packA = np.zeros((128, A_COLS), f32)
    packA[0:Wn, A_W1:A_W1 + E] = w1at
    packA[0:Wn + 1, A_W2:A_W2 + E] = w2bt
    packA[0:Wn + 1, A_Q] = q

    packB_shared = np.zeros((128, B_COLS), f32)
    packB_shared[:, B_SGN] = sgn
    # jh-interleaved: col 2i+jh holds bkkt[jh*128:(jh+1)*128, i]
    packB_shared[:, B_BKT:B_BKT + 2 * K] = np.transpose(
        bkkt.reshape(2, 128, K), (1, 2, 0)).reshape(128, 2 * K)
    packB_shared[0:1, B_ONE:B_ONE + K] = 1.0
    packB_shared[:, B_ID:B_ID + E] = np.eye(128, dtype=f32)
    packB_shared[:, B_FCW:B_FCW + O] = fcw2t[0:128, :]
    packB_shared[:, B_FCW + O:B_FCW + 2 * O] = fcw2t[128:256, :]
    packB_shared[0, B_FCB:B_FCB + O] = fcb2

    in_maps = []
    for c in range(N_CORES):
        pa = packA.copy()
        pb = packB_shared.copy()
        for i in range(BPC):
            xb = x[BPC * c + i]                                  # [64,256]
            xb1 = np.concatenate([xb, np.ones((1, K), f32)], 0)  # [65,256]
            vt = xb.T                                            # [256,64]
            xto2 = np.concatenate([vt, np.full((K, 1), 2.0, f32)], 1)
            col = A_XB0 if i == 0 else A_XB1
            pa[0:Wn + 1, col:col + K] = xb1
            for h in range(2):
                c0 = B_XTO + (2 * i + h) * (Wn + 1)
                pb[:, c0:c0 + Wn + 1] = xto2[128 * h:128 * h + 128, :]
        in_maps.append({
            "packA": np.ascontiguousarray(pa.astype(bf16)),
            "packB": np.ascontiguousarray(pb.astype(bf16)),
        })
    return in_maps


def kernel(x, lin_w, lin_b, a, bias_kk, fc_w, fc_b, _trace=False):
    nc = _get_program()
    in_maps = _host_prep(np.asarray(x), np.asarray(lin_w), np.asarray(lin_b),
                         np.asarray(a), np.asarray(bias_kk),
                         np.asarray(fc_w), np.asarray(fc_b))
    res = run_bass_kernel_spmd(nc, in_maps, list(range(N_CORES)),
                               trace=_trace)
    out = np.empty((B, Wn, O), np.float32)
    for c in range(N_CORES):
        o = res.results[c]["outp"]          # (O, BPC*Wn)
        for i in range(BPC):
            out[BPC * c + i] = o[:, Wn * i:Wn * (i + 1)].T
    if _trace:
        return out, res
    return out


# revision 81
# speedup vs baseline: 1.0015x; 1.0001x over previous
"""Trainium2 Bass kernel for nn_GatFeatDecoder (GAT-style decoder).

Reference computation per batch b (B=16, W=64, K=256, E=128, O=64):
    v = x[b].T                               (K, W)
    l = v @ W1.T ; r = v @ W2.T              (K, E) each
    e[i,j]  = sum_e a_e * LeakyReLU(l[i,e] + r[j,e] + lin_b[e]) + bias_kk[i,j]
    attn    = softmax_j(e)
    h       = sigmoid(attn @ v)              (K, W)
    out[b]  = h.T @ fc_w.T + fc_b            (W, O)

Data-parallel: 2 batches per core on 8 cores, no collectives.

Math folding (per-core), same as the v1 kernel:
  * z~ = (1-a)|a_e| (l+r+b); sum_e a_e LeakyReLU = sum_e sgn_e relu(z~)
    + alpha' * sum_e sgn_e z~, alpha' = alpha/(1-alpha).  The per-i part
    of the linear term cancels in softmax; the per-j part srb_j =
    sum_w q_w xb[w,j] with q = alpha' * W2b @ sgn precomputed on device,
    and enters as the per-partition bias of the exp() activation.
  * bias_kk^T is accumulated into each score PSUM tile by one extra
    matmul with identity weights, so exp() reads PSUM directly.
  * softmax without row-max (logits bounded): P^T = exp(S^T + srb_j).
    attn@v and the denominator come from matmuls with rhs = [v | 2.0];
    h = sigmoid(num/den) = 0.5*(tanh(num * (0.5/den)) + 1) via ACT Tanh
    with per-partition scale = reciprocal(2*sum exp); the 0.5/0.5 affine
    is folded into the fc weights/bias on the host.

v2 score-matmul inversion (the big change vs v1):
  * v1 streamed each relu tile [E,K] through the PE as the MOVING
    operand of a sign-weighted matmul (256 rows -> ~107ns each, a
    54.6us PE floor for 512 tiles).  v2 makes the relu tile the
    STATIONARY operand instead: per query-node i the tile
    T^i[e, j] = relu(rtb[e,j] + lt[e,i]) is produced once, and two
    matmuls (one per j-half chain) contract it against a single sgn
    column as the moving operand, writing one PSUM column
    S^T[jh][:, i].  Output free size is 1, so each matmul costs ~4ns;
    the whole score reduction is ~2us of PE time and the kernel is
    bound by relu-tile PRODUCTION on DVE/ACT/Pool instead
    (134/391/429 ns per [128,256] tile; DVE runs in 4x mode on bf16).
  * Tiles are indexed by i (bias = lt column) rather than j (bias =
    rtb column) so S^T lands in the same [j-half, i] layout v1 used;
    the exp/attn/fc epilogue is unchanged.
  * The PE is now nearly idle, so the p-state ramp is irrelevant and
    v1's warm-up matmul prologue is dropped entirely.
  * Production is split DVE/ACT/Pool ~20/6/6 per 32 tiles (inverse to
    the measured per-tile costs), with the fixed per-engine work (exp
    on ACT, copies on DVE/Pool) folded into the balance.
"""

import numpy as np
import ml_dtypes

import concourse.bass as bass
import concourse.bacc as bacc
import concourse.tile as tile
from concourse import mybir
from concourse.bass_utils import run_bass_kernel_spmd

ALPHA = 0.2
B, Wn, K, E, O = 16, 64, 256, 128, 64
N_CORES = 8
BPC = B // N_CORES  # batches per core

FP32 = mybir.dt.float32
BF16 = mybir.dt.bfloat16
AF = mybir.ActivationFunctionType
ALU = mybir.AluOpType

# ---- packed-constant column layout (bf16, 128 partitions) ----
# pack A (early: needed for projections + first relu tiles)
A_W1 = 0                      # w1at   [64,128]  rows 0:64
A_W2 = A_W1 + E               # w2bt   [65,128]  rows 0:65
A_Q = A_W2 + E                # q      [65,1]
A_XB0 = A_Q + 1               # xb b0  [65,256]  rows 0:65 (row 64 = ones)
A_XB1 = A_XB0 + K             # xb b1  [65,256]
A_COLS = A_XB1 + K

# pack B (epilogue constants; lands while batch-0 tiles stream)
B_SGN = 0                     # sgn column [128,1]
B_XTO = B_SGN + 1             # xto2   4 x [128,65]  (b,h) = [v | 2.0]
B_BKT = B_XTO + 4 * (Wn + 1)  # bkkt^T jh-interleaved [128,512] (col 2i+jh)
B_ID = B_BKT + 2 * K          # identity [128,128]
B_FCW = B_ID + E              # fcw2t  2 x [128,64]
B_FCB = B_FCW + 2 * O         # fcb2 row [1,64] (partition 0)
B_ONE = B_FCB + O             # ones row [1,256] (partition 0)
B_COLS = B_ONE + K

# tile-production engine split per 256 i's: DVE 161, ACT 48, Pool 47
# (largest-remainder interleave; inverse to measured per-tile costs
# 134/391/451ns with the fixed per-engine work folded in)
def _make_pattern(n_v=20, n_a=6, n_p=6, span=32):
    quota = {"V": n_v / span, "A": n_a / span, "P": n_p / span}
    acc = {"V": 0.0, "A": 0.0, "P": 0.0}
    pat = []
    for _ in range(span):
        for k in acc:
            acc[k] += quota[k]
        k = max(acc, key=lambda t: acc[t])
        acc[k] -= 1.0
        pat.append(k)
    return pat

PATTERN = _make_pattern()
# surgical engine swaps: lighten ACT at the very end of batch 1 so its
# queue frees for the epilogue tail sooner (counts shift, arrangement kept)
OVERRIDE = {(1, i): "V" for i in range(248, 256) if PATTERN[i % 32] == "A"}


def _build_program():
    nc = bacc.Bacc("TRN2", target_bir_lowering=False, debug=False,
                   num_devices=N_CORES)

    d_packA = nc.dram_tensor("packA", [128, A_COLS], BF16, kind="ExternalInput")
    d_packB = nc.dram_tensor("packB", [128, B_COLS], BF16, kind="ExternalInput")
    d_out = nc.dram_tensor("outp", [O, BPC * Wn], FP32, kind="ExternalOutput")

    with tile.TileContext(nc) as tc:
        with (
            tc.tile_pool(name="consts", bufs=1) as consts,
            tc.tile_pool(name="setup", bufs=2) as setup,
            tc.tile_pool(name="trelu", bufs=96) as trelu,
            tc.tile_pool(name="etiles", bufs=4) as etiles,
            tc.tile_pool(name="small", bufs=8) as small,
            tc.tile_pool(name="psA", bufs=2, space="PSUM") as psA,
            tc.tile_pool(name="psS", bufs=2, space="PSUM") as psS,
            tc.tile_pool(name="psM", bufs=2, space="PSUM") as psM,
            tc.tile_pool(name="psN", bufs=2, space="PSUM") as psN,
        ):
            # separate tiles per DMA chunk so consumers depend only on the
            # chunk they read (whole-tile dep tracking would make the
            # batch-0 projections wait for xb1's DMA too)
            cA1 = consts.tile([128, A_XB1], BF16, tag="cA1")
            nc.sync.dma_start(out=cA1[:], in_=d_packA.ap()[:, 0:A_XB1])
            cA2 = consts.tile([128, K], BF16, tag="cA2")
            nc.sync.dma_start(out=cA2[:], in_=d_packA.ap()[:, A_XB1:A_COLS])
            cB = consts.tile([128, B_COLS], BF16, tag="cB")
            # SP HWDGE (third in queue): keeps the descriptor-gen work off
            # the Pool engine, which is a production bottleneck
            nc.sync.dma_start(out=cB[:], in_=d_packB.ap())

            w1at_v = cA1[0:Wn, A_W1:A_W1 + E]
            w2bt_v = cA1[0:Wn + 1, A_W2:A_W2 + E]
            q_v = cA1[0:Wn + 1, A_Q:A_Q + 1]

            def xb_v(b):
                if b == 0:
                    return cA1[0:Wn + 1, A_XB0:A_XB0 + K]
                return cA2[0:Wn + 1, 0:K]

            sgn_v = cB[:, B_SGN:B_SGN + 1]

            def xto_v(b, h):
                c = B_XTO + (2 * b + h) * (Wn + 1)
                return cB[:, c:c + Wn + 1]

            bkt_v = cB[:, B_BKT:B_BKT + 2 * K]   # jh-interleaved
            ones_v = cB[0:1, B_ONE:B_ONE + K]

            ident_v = cB[:, B_ID:B_ID + E]

            def fcw_v(ih):
                return cB[:, B_FCW + ih * O:B_FCW + (ih + 1) * O]

            fcb_row = cB[0:1, B_FCB:B_FCB + O]
            ot2 = consts.tile([O, BPC * Wn], FP32, tag="ot2")

            # per-batch state
            lt_f = [None] * BPC     # fp32 [E,K]  (scalar/bias source, per-i)
            rtb_b = [None] * BPC    # bf16 [E,K]  (tile in0)
            srb_row = [None] * BPC  # bf16 [1,K]  (srb as a partition-0 row)
            ps_sc = [None] * BPC    # [K//2, 2K] score PSUM, col = 2i+jh
            pT = [None] * BPC       # [K//2, 2K] exp(scores), same layout
            tt = [[None, None] for _ in range(BPC)]
            ps_mix_t = [None] * BPC  # [:,0:65] attn ih0 | [0:64,68:132] fc
            #                          [0:1,132:388] srb row

            def emit_proj(b, chunk_lt=False):
                ps_lr = psA.tile([E, 2 * K], FP32, tag="ps_lr",
                                 name=f"ps_lr{b}")
                # lt matmul first: the 32-column lt chunk plus rtb are
                # what the first tiles need; this order lets the lt chunk
                # copy overlap the rtb matmul
                nc.tensor.matmul(ps_lr[:, 0:K], w1at_v, xb_v(b)[0:Wn, :],
                                 start=True, stop=True)
                nc.tensor.matmul(ps_lr[:, K:2 * K], w2bt_v, xb_v(b),
                                 start=True, stop=True)
                lt_f[b] = setup.tile([E, K], FP32, tag="lt_f", name=f"lt_f{b}")
                rtb_b[b] = setup.tile([E, K], BF16, tag="rtb_b",
                                      name=f"rtb_b{b}")
                # batch-0 copies on DVE: rtb first (it gates every tile),
                # then only the first 32 lt columns -- the rest stream in
                # between the first tiles (emit_lt_chunk).  Batch-1 copies
                # go on ACT: the scheduler hoists them into ACT's prologue
                # idle (waiting for the act-table load) instead of delaying
                # DVE's first tile.  (Pool cannot read PSUM at all.)
                if chunk_lt:
                    nc.vector.tensor_copy(lt_f[b][:, 0:32], ps_lr[:, 0:32])
                    nc.vector.tensor_copy(rtb_b[b][:], ps_lr[:, K:2 * K])
                else:
                    nc.scalar.copy(rtb_b[b][:], ps_lr[:, K:2 * K])
                    nc.scalar.copy(lt_f[b][:], ps_lr[:, 0:K])
                # srb row: srb[j] = sum_w q[w] xb[w, j], produced as a
                # [1,256] partition-0 row so it can seed the score PSUM via
                # two rank-1 matmuls (emit_gen_open) -- the exp() then needs
                # no per-partition bias and covers both jh in one instr
                ps_mix_t[b] = psM.tile([128, 390], FP32, tag="ps_mix",
                                       name=f"ps_mix{b}")
                nc.tensor.matmul(ps_mix_t[b][0:1, 132:132 + K], q_v,
                                 xb_v(b), start=True, stop=True)
                srb_row[b] = setup.tile([1, K], BF16, tag="srb_row",
                                        name=f"srb_row{b}")
                nc.scalar.copy(srb_row[b][:], ps_mix_t[b][0:1, 132:132 + K])
                return ps_lr

            def emit_lt_chunk(b, ps_lr, c0, c1):
                nc.vector.tensor_copy(lt_f[b][:, c0:c1], ps_lr[:, c0:c1])

            def emit_gen_open(b):
                ps_sc[b] = psS.tile([K // 2, 2 * K], FP32, tag="ps_sc",
                                    name=f"ps_sc{b}")
                # open both chains at once: bias_kk (jh-interleaved) via
                # identity weights, then srb[j] broadcast along i via one
                # rank-1 matmul per jh (stride-2 output columns)
                nc.tensor.matmul(ps_sc[b][:], ident_v, bkt_v,
                                 start=True, stop=False,
                                 skip_group_check=True)
                for jh in range(2):
                    nc.tensor.matmul(
                        ps_sc[b][:, jh:2 * K:2],
                        srb_row[b][0:1, 128 * jh:128 * jh + 128], ones_v,
                        start=False, stop=False, skip_group_check=True)

            def emit_gen(b, i0, i1):
                for i in range(i0, i1):
                    eng = OVERRIDE.get((b, i)) or PATTERN[i % 32]
                    bias_c = lt_f[b][:, i:i + 1]
                    tj = trelu.tile([E, K], BF16, tag="tj")
                    if eng == "A":
                        nc.scalar.activation(
                            tj[:], rtb_b[b][:], AF.Relu, bias=bias_c,
                            scale=1.0)
                    elif eng == "P":
                        nc.gpsimd.tensor_scalar(
                            out=tj[:], in0=rtb_b[b][:],
                            scalar1=bias_c, scalar2=0.0,
                            op0=ALU.add, op1=ALU.max)
                    else:
                        nc.vector.tensor_scalar(
                            out=tj[:], in0=rtb_b[b][:],
                            scalar1=bias_c, scalar2=0.0,
                            op0=ALU.add, op1=ALU.max)
                    for jh in range(2):
                        nc.tensor.matmul(
                            ps_sc[b][:, 2 * i + jh:2 * i + jh + 1],
                            tj[:, 128 * jh:128 * jh + 128], sgn_v,
                            start=False, stop=(i == K - 1),
                            skip_group_check=True)

            def emit_exp_chunk(b, c0, c1):
                # columns [c0, c1) of both jh chains (interleaved) are
                # final as soon as their col-matmuls have landed; srb is
                # already in PSUM, so one bias-free exp covers both jh
                if pT[b] is None:
                    pT[b] = etiles.tile([K // 2, 2 * K], BF16, tag="pT",
                                        name=f"pT{b}")
                nc.scalar.activation(pT[b][:, 2 * c0:2 * c1],
                                     ps_sc[b][:, 2 * c0:2 * c1], AF.Exp,
                                     scale=1.0)

            attn_num = [[None, None] for _ in range(BPC)]

            def emit_attn_mm(b, ih):
                if ih == 0:
                    num = ps_mix_t[b][:, 0:Wn + 1]
                else:
                    num = psN.tile([K // 2, Wn + 1], FP32, tag="ps_att1",
                                   name=f"ps_att1{b}")
                attn_num[b][ih] = num
                for jh in range(2):
                    nc.tensor.matmul(
                        num,
                        pT[b][:, 256 * ih + jh:256 * ih + jh + 255:2],
                        xto_v(b, jh), start=(jh == 0), stop=(jh == 1))

            def emit_attn_fin(b, ih):
                num = attn_num[b][ih]
                rcol = small.tile([K // 2, 1], FP32, tag=f"rcol{ih}",
                                  name=f"rcol{b}_{ih}")
                nc.vector.reciprocal(rcol[:], num[:, Wn:Wn + 1])
                tt[b][ih] = small.tile([K // 2, Wn], BF16, tag=f"tt{ih}",
                                       name=f"tt{b}_{ih}")
                nc.scalar.activation(tt[b][ih][:], num[:, 0:Wn],
                                     AF.Tanh, scale=rcol[:])

            def emit_fc_bias(b):
                # seed the fc accumulation with fcb (rank-1: fcb ⊗ ones) so
                # no bias-add instruction sits between fc and the out-DMA
                nc.tensor.matmul(ps_mix_t[b][0:O, 68:68 + Wn], fcb_row,
                                 cB[0:1, B_ONE:B_ONE + Wn],
                                 start=True, stop=False,
                                 skip_group_check=True)

            def emit_fc_mm(b):
                ps_o = ps_mix_t[b][0:O, 68:68 + Wn]
                for ih in range(2):
                    nc.tensor.matmul(ps_o, fcw_v(ih), tt[b][ih][:],
                                     start=False, stop=(ih == 1),
                                     skip_group_check=True)

            def emit_fc_out(b, on_act=False):
                # pure PSUM->SBUF copy (fcb already seeded via emit_fc_bias)
                if on_act:
                    nc.scalar.copy(ot2[:, Wn * b:Wn * (b + 1)],
                                   ps_mix_t[b][0:O, 68:68 + Wn])
                else:
                    nc.vector.tensor_copy(ot2[:, Wn * b:Wn * (b + 1)],
                                          ps_mix_t[b][0:O, 68:68 + Wn])

            # software-pipelined emission: epilogue pieces are interleaved
            # into the tile stream so in-order engine queues never block on
            # a not-yet-ready epilogue instruction (emission position is
            # queue position), and only a short chain trails the last tile
            ps_lr0 = emit_proj(0, chunk_lt=True)
            emit_gen_open(0)
            emit_gen(0, 0, 8)
            emit_lt_chunk(0, ps_lr0, 32, 64)
            emit_gen(0, 8, 20)
            emit_lt_chunk(0, ps_lr0, 64, 128)
            emit_gen(0, 20, 40)
            emit_lt_chunk(0, ps_lr0, 128, K)
            emit_gen(0, 40, 96)
            emit_proj(1)
            emit_gen(0, 96, 132)
            emit_exp_chunk(0, 0, 128)
            emit_gen(0, 132, 140)
            emit_attn_mm(0, 0)
            emit_fc_bias(0)
            emit_gen(0, 140, 164)
            emit_attn_fin(0, 0)
            emit_gen(0, 164, K)
            emit_gen_open(1)
            emit_gen(1, 0, 4)
            emit_exp_chunk(0, 128, 240)
            emit_gen(1, 4, 8)
            emit_exp_chunk(0, 240, K)   # b0 tail cols
            emit_gen(1, 8, 12)
            emit_attn_mm(0, 1)
            emit_gen(1, 12, 36)
            emit_attn_fin(0, 1)
            emit_gen(1, 36, 44)
            emit_fc_mm(0)
            emit_fc_out(0)
            nc.sync.dma_start(out=d_out.ap()[:, 0:Wn], in_=ot2[:, 0:Wn])
            emit_gen(1, 44, 132)
            emit_exp_chunk(1, 0, 128)
            emit_gen(1, 132, 140)
            emit_attn_mm(1, 0)
            emit_fc_bias(1)
            emit_gen(1, 140, 164)
            emit_attn_fin(1, 0)
            emit_gen(1, 164, 196)
            emit_exp_chunk(1, 128, 192)
            emit_gen(1, 196, 244)
            emit_exp_chunk(1, 192, 240)
            emit_gen(1, 244, K)
            emit_exp_chunk(1, 240, K)   # b1 tail chunk (tiny)
            emit_attn_mm(1, 1)
            emit_attn_fin(1, 1)
            emit_fc_mm(1)
            emit_fc_out(1, on_act=True)
            nc.sync.dma_start(out=d_out.ap()[:, Wn:2 * Wn],
                              in_=ot2[:, Wn:2 * Wn])

    nc.compile()
    return nc


_NC_CACHE = {}


def _get_program():
    if "nc" not in _NC_CACHE:
        _NC_CACHE["nc"] = _build_program()
    return _NC_CACHE["nc"]


def _host_prep(x, lin_w, lin_b, a, bias_kk, fc_w, fc_b):
    f32 = np.float32
    bf16 = ml_dtypes.bfloat16
    x = np.ascontiguousarray(x, f32)
    aa = (np.abs(a) * (1.0 - ALPHA)).astype(f32)
    sgn = np.sign(a).astype(f32)
    alpha_p = ALPHA / (1.0 - ALPHA)

    w1at = (lin_w[:, :Wn] * aa[:, None]).T.astype(f32)          # [64,128]
    w2t = (lin_w[:, Wn:] * aa[:, None]).T                        # [64,128]
    bt = (lin_b * aa)[None, :]
    w2bt = np.concatenate([w2t, bt], 0).astype(f32)              # [65,128]
    q = (alpha_p * (w2bt @ sgn)).astype(f32)                     # [65]
    bkkt = bias_kk.T.astype(f32)                                 # [256,256]
    fcw2t = (0.5 * fc_w).T.astype(f32)                           # [256,64]
    fcb2 = (fc_b + 0.5 * fc_w.sum(1)).astype(f32)                # [64]

    packA = np.zeros((128, A_COLS), f32)
    packA[0:Wn, A_W1:A_W1 + E] = w1at
    packA[0:Wn + 1, A_W2:A_W2 + E] = w2bt
    packA[0:Wn + 1, A_Q] = q

    packB_shared = np.zeros((128, B_COLS), f32)
    packB_shared[:, B_SGN] = sgn
    # jh-interleaved: col 2i+jh holds bkkt[jh*128:(jh+1)*128, i]
    packB_shared[:, B_BKT:B_BKT + 2 * K] = np.transpose(
        bkkt.reshape(2, 128, K), (1, 2, 0)).reshape(128, 2 * K)
    packB_shared[0:1, B_ONE:B_ONE + K] = 1.0
    packB_shared[:, B_ID:B_ID + E] = np.eye(128, dtype=f32)
    packB_shared[:, B_FCW:B_FCW + O] = fcw2t[0:128, :]
    packB_shared[:, B_FCW + O:B_FCW + 2 * O] = fcw2t[128:256, :]
    packB_shared[0, B_FCB:B_FCB + O] = fcb2

    in_maps = []
    for c in range(N_CORES):
        pa = packA.copy()
        pb = packB_shared.copy()
        for i in range(BPC):
            xb = x[BPC * c + i]                                  # [64,256]
            xb1 = np.concatenate([xb, np.ones((1, K), f32)], 0)  # [65,256]
            vt = xb.T                                            # [256,64]
            xto2 = np.concatenate([vt, np.full((K, 1), 2.0, f32)], 1)
            col = A_XB0 if i == 0 else A_XB1
            pa[0:Wn + 1, col:col + K] = xb1
            for h in range(2):
                c0 = B_XTO + (2 * i + h) * (Wn + 1)
                pb[:, c0:c0 + Wn + 1] = xto2[128 * h:128 * h + 128, :]
        in_maps.append({
            "packA": np.ascontiguousarray(pa.astype(bf16)),
            "packB": np.ascontiguousarray(pb.astype(bf16)),
        })
    return in_maps


def kernel(x, lin_w, lin_b, a, bias_kk, fc_w, fc_b, _trace=False):
    nc = _get_program()
    in_maps = _host_prep(np.asarray(x), np.asarray(lin_w), np.asarray(lin_b),
                         np.asarray(a), np.asarray(bias_kk),
                         np.asarray(fc_w), np.asarray(fc_b))
    res = run_bass_kernel_spmd(nc, in_maps, list(range(N_CORES)),
                               trace=_trace)
    out = np.empty((B, Wn, O), np.float32)
    for c in range(N_CORES):
        o = res.results[c]["outp"]          # (O, BPC*Wn)
        for i in range(BPC):
            out[BPC * c + i] = o[:, Wn * i:Wn * (i + 1)].T
    if _trace:
        return out, res
    return out
